# revision 5
# baseline (speedup 1.0000x reference)
"""AZConv2d Trainium2 kernel — W-major banded-matmul stencil design.

Math (per batch, from the reference):
  mu = softmax_r(gate_w @ x + gate_b)                      [4, L]
  alpha[r,s,l] = mu[r,l] * mu[r,l+d_s] * kern[r,s]
  agg[(r,c),l] = sum_s alpha[r,s,l]/asum[l] * x[c,l+d_s]
  out = pw_w @ agg + pw_b

Identity: with mu = E/Z the center 1/Z cancels between numerator and
normalizer:
  out[o,l] = sum_r ehat[r,l] * (pw_r @ conv3x3(mu_r*x, kern_r))[o,l]
  ehat = E / sum_r E_r * conv3x3(mu_r)            (Z-free)

Layouts:
  * W-major [w=128 interior cols on partitions, free=(h, c)]: the 3x3
    conv becomes 3 accumulating matmuls with tridiagonal band weights
    (dx mixing via the 128x128 band, dy via +-64 free offsets). All
    per-pixel normalization tensors are [128, 66] tiles.
  * Gate (1x1 conv) runs C-major; exp(+bias) on the Act engine writes
    E (interior w) which one XBAR DMA-transpose converts to W-major.
  * conv results are scaled by ehat into D[w,(h,c)] (bf16), and an
    XBAR DMA-transpose yields T[(h%2,c), (h//2, w)] whose 64-partition
    parity slices feed the pointwise matmul directly; bias enters as a
    rank-1 matmul and outputs DMA straight from PSUM.

Sharding: batch B=8 -> one batch per NeuronCore. Image processed in two
H-halves (66 padded rows: halo + 64 + halo).
"""

import numpy as np

import concourse.bass as bass
import concourse.bacc as bacc
import concourse.mybir as mybir
import concourse.tile as tile
from concourse.bass_utils import run_bass_kernel_spmd

# ---- problem constants (hardcoded per contract) ----
B, C, H, W = 8, 64, 128, 128
R, COUT = 4, 128
PW_, PH_ = 130, 130            # padded image
HB = 66                        # padded rows per half (1 halo + 64 + 1 halo)
GL = HB * PW_                  # 8580 gate pixels per half (padded w)
FH = HB * C                    # 4224 = free size of a W-major half (h, c)
NM = 64                        # stencil margin (one h step = 64 elems)
NM2 = 8                        # nu margin
GCH = 260                      # gate chunk = 2 padded rows
NG = GL // GCH                 # 33
SCH = 384                      # stencil chunk = 6 h-blocks
NST = FH // SCH                # 11
NB = 33                        # h-pair blocks in T layout
PWB = 4                        # b-blocks per pw chunk
NPW = 9                        # ceil(33/4)

BF = mybir.dt.bfloat16
F32 = mybir.dt.float32

_CACHED = {}


def _build():
    nc = bacc.Bacc(None, target_bir_lowering=False)
    x_cm = nc.dram_tensor("x_cm", [C, PH_ * PW_], BF, kind="ExternalInput")
    x_wm = nc.dram_tensor("x_wm", [W, PH_ * C], BF, kind="ExternalInput")
    gwh = nc.dram_tensor("gwh", [C, R], BF, kind="ExternalInput")
    gwl = nc.dram_tensor("gwl", [C, R], BF, kind="ExternalInput")
    gb = nc.dram_tensor("gb", [R, 1], F32, kind="ExternalInput")
    bands = nc.dram_tensor("bands", [W, 12 * W], BF, kind="ExternalInput")
    pwt = nc.dram_tensor("pwt", [W, R * COUT], BF, kind="ExternalInput")
    pwb = nc.dram_tensor("pwb", [1, COUT], BF, kind="ExternalInput")
    onesv = nc.dram_tensor("onesv", [1, PWB * W], BF, kind="ExternalInput")
    y = nc.dram_tensor("y", [COUT, H * W], F32, kind="ExternalOutput")

    with tile.TileContext(nc) as tc:
        with (
            tc.tile_pool(name="consts", bufs=1) as consts,
            tc.tile_pool(name="fat", bufs=1) as fat,
            tc.tile_pool(name="maps", bufs=1) as maps,
            tc.tile_pool(name="outc", bufs=3) as outp,
            tc.tile_pool(name="ps_s", bufs=2, space="PSUM") as ps_gp,
            tc.tile_pool(name="ps_st", bufs=4, space="PSUM") as ps_stp,
            tc.tile_pool(name="ps_pw", bufs=2, space="PSUM") as ps_pwp,
        ):
            c_gwh = consts.tile([C, R], BF, name="c_gwh")
            c_gwl = consts.tile([C, R], BF, name="c_gwl")
            c_gb = consts.tile([R, 1], F32, name="c_gb")
            c_bands = consts.tile([W, 12 * W], BF, name="c_bands")
            c_pwt = consts.tile([W, R * COUT], BF, name="c_pwt")
            c_pwb = consts.tile([1, COUT], BF, name="c_pwb")
            c_ones = consts.tile([1, PWB * W], BF, name="c_ones")
            for t, d in [
                (c_gwh, gwh), (c_gwl, gwl), (c_gb, gb), (c_bands, bands),
                (c_pwt, pwt), (c_pwb, pwb), (c_ones, onesv),
            ]:
                nc.sync.dma_start(out=t, in_=d[:, :])

            for half in range(2):
                h0 = half * 64            # first padded row of this half
                XCM = fat.tile([C, GL], BF, name="XCM")
                XWM = fat.tile([W, FH], BF, name="XWM")
                E16 = fat.tile([16, HB * W], BF, name="E16")
                XR = [fat.tile([W, NM + FH + NM], BF, name=f"XR{r}")
                      for r in range(R)]
                D = [fat.tile([W, FH], BF, name=f"D{r}") for r in range(R)]
                T = [fat.tile([W, FH], BF, name=f"T{r}") for r in range(R)]
                EW = maps.tile([W, HB * 16], BF, name="EW")
                NU = maps.tile([W, NM2 + R * HB + NM2], BF, name="NU")
                EHAT = maps.tile([W, R * HB], BF, name="EHAT")
                EV = maps.tile([W, R * HB], BF, name="EV")
                ZA = maps.tile([W, HB], BF, name="ZA")
                ZB = maps.tile([W, HB], BF, name="ZB")
                ZS = maps.tile([W, HB], BF, name="ZS")
                ZI = maps.tile([W, HB], BF, name="ZI")
                ASI = maps.tile([W, HB], BF, name="ASI")

                # ---- loads ----
                nc.sync.dma_start(
                    out=XCM,
                    in_=bass.AP(tensor=x_cm, offset=h0 * PW_,
                                ap=[[PH_ * PW_, C], [1, GL]]))
                nc.sync.dma_start(
                    out=XWM,
                    in_=bass.AP(tensor=x_wm, offset=h0 * C,
                                ap=[[PH_ * C, W], [1, FH]]))

                # ---- gate: logits -> E = exp(logits + b), interior w ----
                E16v = E16.rearrange("p (h w) -> p h w", w=W)
                for g in range(NG):
                    a = g * GCH
                    ps = ps_gp.tile([R, GCH], F32, name="ps_g", tag="g")
                    nc.tensor.matmul(ps, c_gwh, XCM[:, a:a + GCH],
                                     start=True, stop=False)
                    nc.tensor.matmul(ps, c_gwl, XCM[:, a:a + GCH],
                                     start=False, stop=True)
                    psv = ps.rearrange("p (h w) -> p h w", w=PW_)
                    nc.scalar.activation(
                        out=E16v[0:R, 2 * g:2 * g + 2, :],
                        in_=psv[:, :, 1:129],
                        func=mybir.ActivationFunctionType.Exp,
                        bias=c_gb, scale=1.0)

                # ---- E -> W-major: EW[w, (h, j)] = E16[j, 128h + w] ----
                nc.sync.dma_start_transpose(
                    out=EW.rearrange("p (b j) -> p b j", j=16), in_=E16)

                def esl(r):
                    return bass.AP(tensor=EW.tensor, offset=EW.offset + r,
                                   ap=[list(EW.ap[0]), [16, HB]])

                # ---- Z and nu ----
                mul = mybir.AluOpType.mult
                add = mybir.AluOpType.add
                nc.vector.tensor_tensor(out=ZA, in0=esl(0), in1=esl(1), op=add)
                nc.vector.tensor_tensor(out=ZB, in0=esl(2), in1=esl(3), op=add)
                nc.vector.tensor_tensor(out=ZS, in0=ZA, in1=ZB, op=add)
                with nc.allow_low_precision(reason="bf16 pipeline"):
                    nc.vector.reciprocal(ZI, ZS)
                nc.vector.memset(NU[:, 0:NM2], 0.0)
                nc.vector.memset(NU[:, NM2 + R * HB:], 0.0)
                for r in range(R):
                    nc.vector.tensor_tensor(
                        out=NU[:, NM2 + r * HB:NM2 + (r + 1) * HB],
                        in0=esl(r), in1=ZI, op=mul)
                # zero nu at the true pad row of this half
                pad_h = 0 if half == 0 else HB - 1
                nc.vector.memset(
                    bass.AP(tensor=NU.tensor,
                            offset=NU.offset + NM2 + pad_h,
                            ap=[list(NU.ap[0]), [HB, R]]), 0.0)

                # ---- V_r = conv3x3(nu_r); AS = sum_r E_r*V_r; ehat ----
                for r in range(R):
                    ps_v = ps_gp.tile([W, HB], F32, name="ps_v", tag="g")
                    for j, dy in enumerate((-1, 0, 1)):
                        bsl = c_bands[:, (3 * r + j) * W:(3 * r + j + 1) * W]
                        nc.tensor.matmul(
                            ps_v, bsl,
                            NU[:, NM2 + r * HB + dy:NM2 + (r + 1) * HB + dy],
                            start=(j == 0), stop=(j == 2))
                    nc.vector.tensor_tensor(
                        out=EV[:, r * HB:(r + 1) * HB],
                        in0=esl(r), in1=ps_v, op=mul)
                nc.vector.tensor_tensor(out=ZA, in0=EV[:, 0:HB],
                                        in1=EV[:, HB:2 * HB], op=add)
                nc.vector.tensor_tensor(out=ZB, in0=EV[:, 2 * HB:3 * HB],
                                        in1=EV[:, 3 * HB:4 * HB], op=add)
                nc.vector.tensor_tensor(out=ZS, in0=ZA, in1=ZB, op=add)
                with nc.allow_low_precision(reason="bf16 pipeline"):
                    nc.vector.reciprocal(ASI, ZS)
                for r in range(R):
                    nc.vector.tensor_tensor(
                        out=EHAT[:, r * HB:(r + 1) * HB],
                        in0=esl(r), in1=ASI, op=mul)

                # ---- X_r = x * nu_r (c-broadcast), with zero margins ----
                XWMv = XWM.rearrange("p (h c) -> p h c", c=C)
                for r in range(R):
                    nc.vector.memset(XR[r][:, 0:NM], 0.0)
                    nc.vector.memset(XR[r][:, NM + FH:], 0.0)
                    nc.vector.tensor_tensor(
                        out=XR[r][:, NM:NM + FH].rearrange(
                            "p (h c) -> p h c", c=C),
                        in0=XWMv,
                        in1=bass.AP(tensor=NU.tensor,
                                    offset=NU.offset + NM2 + r * HB,
                                    ap=[list(NU.ap[0]), [1, HB], [0, C]]),
                        op=mul)

                # ---- stencil: 3 band matmuls per chunk; D = ehat * conv ----
                for r in range(R):
                    for ci in range(NST):
                        a = ci * SCH
                        ps_c = ps_stp.tile([W, SCH], F32, name="ps_c", tag="s")
                        for j, dy in enumerate((-1, 0, 1)):
                            bsl = c_bands[:, (3 * r + j) * W:
                                          (3 * r + j + 1) * W]
                            nc.tensor.matmul(
                                ps_c, bsl,
                                XR[r][:, NM + a + dy * C:
                                      NM + a + dy * C + SCH],
                                start=(j == 0), stop=(j == 2))
                        nc.vector.tensor_tensor(
                            out=D[r][:, a:a + SCH].rearrange(
                                "p (h c) -> p h c", c=C),
                            in0=ps_c.rearrange("p (h c) -> p h c", c=C),
                            in1=bass.AP(
                                tensor=EHAT.tensor,
                                offset=EHAT.offset + r * HB + a // C,
                                ap=[list(EHAT.ap[0]), [1, SCH // C], [0, C]]),
                            op=mul)

                # ---- T[p,(b,w)] = D[w, 128b+p]; p=(h%2)*64+c, b=h//2 ----
                for r in range(R):
                    nc.sync.dma_start_transpose(
                        out=T[r].rearrange("p (b w) -> p b w", w=W),
                        in_=D[r])

                # ---- pointwise: out = sum_r pw_r @ T_r[par] + pw_b ----
                for ci in range(NPW):
                    b0 = ci * PWB
                    nb = min(PWB, NB - b0)
                    fln = nb * W
                    for par in range(2):
                        ps_y = ps_pwp.tile([COUT, fln], F32, name="ps_y",
                                           tag="y")
                        for r in range(R):
                            nc.tensor.matmul(
                                ps_y,
                                c_pwt[par * C:(par + 1) * C,
                                      r * COUT:(r + 1) * COUT],
                                T[r][par * C:(par + 1) * C,
                                     b0 * W:b0 * W + fln],
                                start=(r == 0), stop=False)
                        nc.tensor.matmul(ps_y, c_pwb, c_ones[0:1, 0:fln],
                                         start=False, stop=True)
                        # export rows: local h = 2b+par in [1, 64]
                        bs = max(b0, 1 - par)
                        be = min(b0 + nb - 1, 32 - par)
                        if bs > be:
                            continue
                        n = be - bs + 1
                        oc = outp.tile([COUT, n * W], F32, name="oc")
                        src = bass.AP(
                            tensor=ps_y.tensor,
                            offset=ps_y.offset + (bs - b0) * W,
                            ap=[list(ps_y.ap[0]), [1, n * W]])
                        nc.scalar.activation(
                            out=oc, in_=src,
                            func=mybir.ActivationFunctionType.Copy,
                            scale=1.0)
                        row0 = h0 + 2 * bs + par - 1
                        dst = bass.AP(
                            tensor=y, offset=row0 * W,
                            ap=[[H * W, COUT], [2 * W, n], [1, W]])
                        nc.sync.dma_start(
                            out=dst,
                            in_=bass.AP(tensor=oc.tensor, offset=oc.offset,
                                        ap=[list(oc.ap[0]), [W, n], [1, W]]))
    nc.compile()
    return nc


def _host_prep(inputs):
    import ml_dtypes
    x = np.asarray(inputs["x"], np.float32)
    gate_w = np.asarray(inputs["gate_w"], np.float32)
    gate_b = np.asarray(inputs["gate_b"], np.float32)
    theta = np.asarray(inputs["theta"], np.float32)
    rsu = np.asarray(inputs["raw_sigma_u"], np.float32)
    rss = np.asarray(inputs["raw_sigma_s"], np.float32)
    pw_w = np.asarray(inputs["pw_w"], np.float32)
    pw_b = np.asarray(inputs["pw_b"], np.float32)

    tobf = lambda a: np.ascontiguousarray(a, np.float32).astype(
        ml_dtypes.bfloat16)

    grid = np.arange(3, dtype=np.float32) - 1.0
    dy = np.repeat(grid, 3)
    dx = np.tile(grid, 3)
    ct, st = np.cos(theta)[:, None], np.sin(theta)[:, None]
    pu = ct * dx[None, :] + st * dy[None, :]
    ps = -st * dx[None, :] + ct * dy[None, :]
    su = (np.log1p(np.exp(rsu)) + 1e-4)[:, None]
    ss = (np.log1p(np.exp(rss)) + 1e-4)[:, None]
    kern = np.exp(-pu ** 2 / su ** 2 - ps ** 2 / ss ** 2)  # [R, 9]

    # C-major padded x (for gate)
    xp = np.zeros((B, C, PH_, PW_), np.float32)
    xp[:, :, 1:129, 1:129] = x
    xp = xp.reshape(B, C, PH_ * PW_)
    # W-major x: x_wm[w, h_pad, c] = x[c, h_pad-1, w]
    xw = np.zeros((B, W, PH_, C), np.float32)
    xw[:, :, 1:129, :] = x.transpose(0, 3, 2, 1)
    xw = xw.reshape(B, W, PH_ * C)

    # band matrices: bands[(r,dy)][w', w] = kern[r, (dy+1)*3 + (dx+1)],
    # dx = w' - w
    bands = np.zeros((W, 12 * W), np.float32)
    for r in range(R):
        for j in range(3):
            blk = np.zeros((W, W), np.float32)
            for dxi in (-1, 0, 1):
                s = j * 3 + (dxi + 1)
                v = kern[r, s]
                for w in range(W):
                    wp = w + dxi
                    if 0 <= wp < W:
                        blk[wp, w] = v
            bands[:, (3 * r + j) * W:(3 * r + j + 1) * W] = blk

    pwt = np.zeros((W, R * COUT), np.float32)
    for r in range(R):
        blk = pw_w[:, r * C:(r + 1) * C].T          # [64, 128]
        pwt[0:C, r * COUT:(r + 1) * COUT] = blk
        pwt[C:2 * C, r * COUT:(r + 1) * COUT] = blk

    def hilo(a):
        hi = np.asarray(a, np.float32).astype(ml_dtypes.bfloat16)
        lo = (np.asarray(a, np.float32)
              - hi.astype(np.float32)).astype(ml_dtypes.bfloat16)
        return hi, lo

    gw_hi, gw_lo = hilo(gate_w.T)
    common = {
        "gwh": gw_hi, "gwl": gw_lo,
        "gb": gate_b.reshape(R, 1).astype(np.float32),
        "bands": tobf(bands),
        "pwt": tobf(pwt),
        "pwb": tobf(pw_b.reshape(1, COUT)),
        "onesv": tobf(np.ones((1, PWB * W))),
    }
    in_maps = []
    for b in range(B):
        m = dict(common)
        m["x_cm"] = tobf(xp[b])
        m["x_wm"] = tobf(xw[b])
        in_maps.append(m)
    return in_maps


def kernel(**inputs):
    if "nc" not in _CACHED:
        _CACHED["nc"] = _build()
    nc = _CACHED["nc"]
    in_maps = _host_prep(inputs)
    res = run_bass_kernel_spmd(nc, in_maps, core_ids=list(range(B)))
    out = np.stack([res.results[b]["y"].reshape(COUT, H, W)
                    for b in range(B)], axis=0)
    return out.astype(np.float32)
